# revision 19
# baseline (speedup 1.0000x reference)
"""DeepSeek-MoE block on 8 Trainium2 NeuronCores (Bass/Tile).

Sharding: expert-parallel. Each core owns 8 of the 64 routed experts plus a
slice of the 2 shared experts. Every core computes the full gate
(softmax + top-6 threshold) for all 1024 tokens, then runs a masked-dense FFN
over its experts: the per-(token, expert) combine weight is zero for
unselected experts, so no token dispatch is needed. Core outputs are partial
sums; the host unshard is a sum over the 8 partials.

Fixed problem shapes (hardcoded per the harness contract):
  x [2, 512, 512] f32, g_w [64, 512], gate_bias [64],
  w1/w3 [66, 512, 64], w2 [66, 64, 512]; 2 shared + 64 routed, top-6.
"""

import sys

import numpy as np

if "/opt/trn_rl_repo" not in sys.path:
    sys.path.insert(0, "/opt/trn_rl_repo")

import concourse.bass as bass
import concourse.mybir as mybir
import concourse.tile as tile
from concourse import bacc
from concourse.bass_utils import run_bass_kernel_spmd

DIM = 512
INTER = 64
N_SHARED = 2
N_ROUTED = 64
TOPK = 6
B, T = 2, 512
NTOK = B * T                 # 1024 tokens
N_CORES = 8
EXP_PER_CORE = N_ROUTED // N_CORES   # 8 routed experts per core
N_SLOT = EXP_PER_CORE + 2            # + 2 shared-expert slots
N_PAIR = N_SLOT // 2                 # 5 expert pairs
N_TILE = NTOK // 128                 # 8 token tiles of 128
ST = 256                             # supertile token width for the FFN
N_ST = NTOK // ST                    # 4 supertiles
NCK = DIM // 128                     # 4 contraction chunks

F32 = mybir.dt.float32
F32R = mybir.dt.float32r
AF = mybir.ActivationFunctionType
ALU = mybir.AluOpType


def build_nc(silu_native=True):
    """Build the single-core Bass program (SPMD across 8 cores).

    silu_native=False lowers SiLU as Sigmoid+mult (CoreSim has no Silu).
    """
    nc = bacc.Bacc("TRN2", target_bir_lowering=False, debug=False)

    # ---- DRAM I/O (per-core values supplied by the host) ----
    # xt: [128, ck*1024] chunk-major per partition (host pre-layouts)
    xt_d = nc.dram_tensor("xt", [128, NCK * NTOK], F32, kind="ExternalInput")
    gwt_d = nc.dram_tensor("gwt", [128, NCK * N_ROUTED], F32, kind="ExternalInput")
    biasb_d = nc.dram_tensor("biasb", [128, N_TILE * N_ROUTED], F32, kind="ExternalInput")
    w1p_d = nc.dram_tensor("w1p", [128, NCK * N_PAIR * 128], F32R, kind="ExternalInput")
    w3p_d = nc.dram_tensor("w3p", [128, NCK * N_PAIR * 128], F32R, kind="ExternalInput")
    w2p_d = nc.dram_tensor("w2p", [128, N_PAIR * DIM], F32R, kind="ExternalInput")
    rows_sh_d = nc.dram_tensor("rows_sh", [2, NTOK], F32, kind="ExternalInput")
    esel_d = nc.dram_tensor("esel", [N_ROUTED + 2, N_PAIR * 128], F32R, kind="ExternalInput")
    ident_d = nc.dram_tensor("ident", [128, 128], F32, kind="ExternalInput")
    pout_d = nc.dram_tensor("pout", [NTOK, DIM], F32, kind="ExternalOutput")

    with tile.TileContext(nc) as tc:
        with (
            tc.tile_pool(name="const", bufs=1) as cpool,
            tc.tile_pool(name="gate", bufs=1) as gpool,
            tc.tile_pool(name="act", bufs=4) as apool,
            tc.tile_pool(name="psA", bufs=2, space="PSUM") as psA,
            tc.tile_pool(name="psO", bufs=1, space="PSUM") as psO,
        ):
            # ---- persistent SBUF loads; gate-critical ones first on sync ----
            xt_sb = cpool.tile([128, NCK * NTOK], F32, tag="xt")
            for ck in range(NCK):
                nc.sync.dma_start(
                    xt_sb[:, ck * NTOK : (ck + 1) * NTOK],
                    xt_d.ap()[:, ck * NTOK : (ck + 1) * NTOK],
                )
            gwt_sb = cpool.tile([128, NCK * N_ROUTED], F32, tag="gwt")
            nc.sync.dma_start(gwt_sb[:], gwt_d.ap())

            xtr_sb = cpool.tile([128, NCK * NTOK], F32R, tag="xtr")
            for ck in range(NCK):
                nc.scalar.dma_start(
                    xtr_sb[:, ck * NTOK : (ck + 1) * NTOK],
                    xt_d.ap()[:, ck * NTOK : (ck + 1) * NTOK].bitcast(F32R),
                )
            w1p_sb = cpool.tile([128, NCK * N_PAIR * 128], F32R, tag="w1p")
            nc.scalar.dma_start(w1p_sb[:], w1p_d.ap())
            w3p_sb = cpool.tile([128, NCK * N_PAIR * 128], F32R, tag="w3p")
            nc.gpsimd.dma_start(w3p_sb[:], w3p_d.ap())
            w2p_sb = cpool.tile([128, N_PAIR * DIM], F32R, tag="w2p")
            nc.gpsimd.dma_start(w2p_sb[:], w2p_d.ap())

            biasb_sb = cpool.tile([128, N_TILE * N_ROUTED], F32, tag="biasb")
            nc.gpsimd.dma_start(biasb_sb[:], biasb_d.ap())
            esel_sb = cpool.tile([N_ROUTED + 2, N_PAIR * 128], F32R, tag="esel")
            nc.gpsimd.dma_start(esel_sb[:], esel_d.ap())
            ident_sb = cpool.tile([128, 128], F32, tag="ident")
            nc.gpsimd.dma_start(ident_sb[:], ident_d.ap())

            # combine-weight rows: 0..63 routed-transposed (only 0..7 selected
            # by esel), 64..65 the shared-expert token masks
            wt_sb = gpool.tile([N_ROUTED + 2, NTOK], F32R, tag="wt")
            nc.gpsimd.dma_start(
                wt_sb[N_ROUTED : N_ROUTED + 2, :], rows_sh_d.ap().bitcast(F32R)
            )

            # ---- gate matmul, transposed: scoresT[e, t] (fp32, N=512) ----
            scT_sb = gpool.tile([N_ROUTED, NTOK], F32, tag="scT")
            for h in range(NTOK // 512):
                scT_ps = psA.tile([N_ROUTED, 512], F32, tag="h1", name=f"scTps{h}")
                for ck in range(NCK):
                    nc.tensor.matmul(
                        scT_ps[:],
                        gwt_sb[:, ck * N_ROUTED : (ck + 1) * N_ROUTED],
                        xt_sb[:, ck * NTOK + h * 512 : ck * NTOK + (h + 1) * 512],
                        start=(ck == 0),
                        stop=(ck == NCK - 1),
                    )
                nc.scalar.copy(scT_sb[:, h * 512 : (h + 1) * 512], scT_ps[:])

            # transpose to token-partition layout [128, tile, 64]
            scores = gpool.tile([128, N_TILE * N_ROUTED], F32, tag="scores")
            for tt in range(N_TILE):
                tps = psA.tile([128, N_ROUTED], F32, tag="h3", name=f"tps{tt}")
                nc.tensor.transpose(
                    tps[:], scT_sb[:, tt * 128 : (tt + 1) * 128], ident_sb[0:64, 0:64]
                )
                nc.scalar.copy(scores[:, tt * N_ROUTED : (tt + 1) * N_ROUTED], tps[:])

            def b3(t):  # [128, N_TILE*64] -> [128, N_TILE, 64]
                return t.rearrange("p (t e) -> p t e", e=N_ROUTED)

            def bc(t):  # [128, N_TILE] -> broadcast [128, N_TILE, 64]
                return t.unsqueeze(-1).to_broadcast([128, N_TILE, N_ROUTED])

            # softmax over the 64 routed experts (batched over all 8 tiles)
            rmax = gpool.tile([128, N_TILE], F32, tag="rmax")
            nc.vector.tensor_reduce(rmax[:], b3(scores[:]), axis=mybir.AxisListType.X, op=ALU.max)
            shifted = gpool.tile([128, N_TILE * N_ROUTED], F32, tag="shifted")
            nc.vector.tensor_tensor(b3(shifted[:]), b3(scores[:]), bc(rmax[:]), op=ALU.subtract)
            exps = gpool.tile([128, N_TILE * N_ROUTED], F32, tag="exps")
            nc.scalar.activation(exps[:], shifted[:], AF.Exp)
            rsum = gpool.tile([128, N_TILE], F32, tag="rsum")
            nc.vector.tensor_reduce(rsum[:], b3(exps[:]), axis=mybir.AxisListType.X, op=ALU.add)
            rinv = gpool.tile([128, N_TILE], F32, tag="rinv")
            nc.vector.reciprocal(rinv[:], rsum[:])
            probs = gpool.tile([128, N_TILE * N_ROUTED], F32, tag="probs")
            nc.vector.tensor_tensor(b3(probs[:]), b3(exps[:]), bc(rinv[:]), op=ALU.mult)

            # biased scores for selection (host pre-shifts bias so biased > 0)
            biased = gpool.tile([128, N_TILE * N_ROUTED], F32, tag="biased")
            nc.vector.tensor_tensor(biased[:], probs[:], biasb_sb[:], op=ALU.add)
            work = gpool.tile([128, N_TILE * N_ROUTED], F32, tag="work")
            nc.scalar.copy(work[:], biased[:])

            # 6th-largest threshold per token: 6x (max, zero-out-the-max)
            m = None
            for it in range(TOPK):
                m = gpool.tile([128, N_TILE], F32, tag=f"m{it}")
                nc.vector.tensor_reduce(m[:], b3(work[:]), axis=mybir.AxisListType.X, op=ALU.max)
                if it < TOPK - 1:
                    keep = gpool.tile([128, N_TILE * N_ROUTED], F32, tag="keep")
                    nc.vector.tensor_tensor(b3(keep[:]), b3(work[:]), bc(m[:]), op=ALU.is_lt)
                    nc.vector.tensor_tensor(work[:], work[:], keep[:], op=ALU.mult)

            mask = gpool.tile([128, N_TILE * N_ROUTED], F32, tag="mask")
            nc.vector.tensor_tensor(b3(mask[:]), b3(biased[:]), bc(m[:]), op=ALU.is_ge)
            wcomb = gpool.tile([128, N_TILE * N_ROUTED], F32, tag="wcomb")
            nc.vector.tensor_tensor(wcomb[:], probs[:], mask[:], op=ALU.mult)

            # transpose combine weights tile-by-tile into rows 0..63 of wt_sb
            for tt in range(N_TILE):
                wtp = psA.tile([N_ROUTED, 128], F32, tag="h3", name=f"wtp{tt}")
                nc.tensor.transpose(
                    wtp[:], wcomb[:, tt * N_ROUTED : (tt + 1) * N_ROUTED], ident_sb[:]
                )
                nc.scalar.copy(wt_sb[0:N_ROUTED, tt * 128 : (tt + 1) * 128], wtp[:])

            # ---- FFN: 4 supertiles x 5 expert pairs, masked-dense ----
            for st in range(N_ST):
                t0 = st * ST
                outp = [
                    psO.tile([128, DIM], F32, name=f"outp{s}", tag=f"out{s}")
                    for s in range(ST // 128)
                ]
                for p in range(N_PAIR):
                    h1 = psA.tile([128, ST], F32, tag="h1")
                    h3 = psA.tile([128, ST], F32, tag="h3")
                    wb = psA.tile([128, ST], F32, tag="wb")
                    for ck in range(NCK):
                        xck = xtr_sb[:, ck * NTOK + t0 : ck * NTOK + t0 + ST]
                        nc.tensor.matmul(
                            h1[:],
                            w1p_sb[:, (ck * N_PAIR + p) * 128 : (ck * N_PAIR + p + 1) * 128],
                            xck,
                            start=(ck == 0),
                            stop=(ck == NCK - 1),
                        )
                        nc.tensor.matmul(
                            h3[:],
                            w3p_sb[:, (ck * N_PAIR + p) * 128 : (ck * N_PAIR + p + 1) * 128],
                            xck,
                            start=(ck == 0),
                            stop=(ck == NCK - 1),
                        )
                    nc.tensor.matmul(
                        wb[:],
                        esel_sb[:, p * 128 : (p + 1) * 128],
                        wt_sb[:, t0 : t0 + ST],
                        start=True,
                        stop=True,
                    )
                    # ACT evacuates all three PSUM banks so PE never waits on DVE
                    silu = apool.tile([128, ST], F32, tag="silu")
                    if silu_native:
                        nc.scalar.activation(silu[:], h1[:], AF.Silu)
                    else:
                        # CoreSim path: silu = h1 * sigmoid(h1)
                        sg = apool.tile([128, ST], F32, tag="sg")
                        nc.scalar.activation(sg[:], h1[:], AF.Sigmoid)
                        h1s = apool.tile([128, ST], F32, tag="h1s")
                        nc.scalar.copy(h1s[:], h1[:])
                        nc.vector.tensor_tensor(silu[:], sg[:], h1s[:], op=ALU.mult)
                    h3s = apool.tile([128, ST], F32, tag="h3s")
                    nc.scalar.copy(h3s[:], h3[:])
                    wbs = apool.tile([128, ST], F32, tag="wbs")
                    nc.scalar.copy(wbs[:], wb[:])
                    prod = apool.tile([128, ST], F32, tag="prod")
                    nc.gpsimd.tensor_tensor(prod[:], silu[:], h3s[:], op=ALU.mult)
                    aT = apool.tile([128, ST], F32R, tag="aT")
                    nc.vector.tensor_tensor(aT[:], prod[:], wbs[:], op=ALU.mult)
                    for s in range(ST // 128):
                        nc.tensor.matmul(
                            outp[s][:],
                            aT[:, s * 128 : (s + 1) * 128],
                            w2p_sb[:, p * DIM : (p + 1) * DIM],
                            start=(p == 0),
                            stop=(p == N_PAIR - 1),
                        )
                for s in range(ST // 128):
                    osb = apool.tile([128, DIM], F32, tag="osb")
                    nc.scalar.copy(osb[:], outp[s][:])
                    nc.sync.dma_start(
                        pout_d.ap()[t0 + s * 128 : t0 + (s + 1) * 128, :], osb[:]
                    )

    nc.compile()
    return nc


def make_core_inputs(x, g_w, gate_bias, w1, w2, w3):
    """Host-side sharding/layout prep. Returns list of 8 per-core input maps."""
    x = np.ascontiguousarray(np.asarray(x, dtype=np.float32)).reshape(NTOK, DIM)
    g_w = np.asarray(g_w, dtype=np.float32)
    gate_bias = np.asarray(gate_bias, dtype=np.float32)
    w1 = np.asarray(w1, dtype=np.float32)
    w2 = np.asarray(w2, dtype=np.float32)
    w3 = np.asarray(w3, dtype=np.float32)

    # xt host layout: [128 partitions, ck*1024] with xt[p, ck*1024+t] = x[t, ck*128+p]
    xt = np.ascontiguousarray(
        x.T.reshape(NCK, 128, NTOK).transpose(1, 0, 2).reshape(128, NCK * NTOK)
    )
    bias_shift = gate_bias - gate_bias.min() + 1.0      # keep biased scores > 0
    ident = np.eye(128, dtype=np.float32)
    # esel[k, p*128 + j] selects wt row k into broadcast partitions j of pair p:
    # pair p < 4 -> routed rows (2p, 2p+1); pair 4 -> shared rows (64, 65)
    esel = np.zeros((N_ROUTED + 2, N_PAIR * 128), dtype=np.float32)
    for p in range(N_PAIR):
        r0 = 2 * p if p < N_PAIR - 1 else N_ROUTED
        esel[r0, p * 128 : p * 128 + 64] = 1.0
        esel[r0 + 1, p * 128 + 64 : (p + 1) * 128] = 1.0

    in_maps = []
    for c in range(N_CORES):
        mine = list(range(EXP_PER_CORE * c, EXP_PER_CORE * (c + 1)))
        perm = mine + [e for e in range(N_ROUTED) if e not in mine]
        # gwt host layout [128, ck*64]: gwt[p, ck*64+e] = g_w[perm[e], ck*128+p]
        gwt_c = np.ascontiguousarray(
            g_w[perm].T.reshape(NCK, 128, N_ROUTED).transpose(1, 0, 2).reshape(128, -1)
        )
        biasb = np.tile(bias_shift[perm], (128, N_TILE))  # [128, 512]

        # expert slots: 8 routed (global idx 2+e) then the 2 shared experts
        slots = [2 + e for e in mine] + [0, 1]
        w1s = w1[slots]                                  # [10, 512, 64]
        w3s = w3[slots]
        w2s = w2[slots]                                  # [10, 64, 512]
        # pair p = slots (2p, 2p+1) concatenated along the inter axis
        w1pair = np.stack(
            [np.concatenate([w1s[2 * p], w1s[2 * p + 1]], axis=1) for p in range(N_PAIR)]
        )  # [5, 512, 128]
        w3pair = np.stack(
            [np.concatenate([w3s[2 * p], w3s[2 * p + 1]], axis=1) for p in range(N_PAIR)]
        )
        w2pair = np.stack(
            [np.concatenate([w2s[2 * p], w2s[2 * p + 1]], axis=0) for p in range(N_PAIR)]
        )  # [5, 128, 512]

        # SBUF layouts: w1p [128p, ck, pair, 128], w2p [128p, pair*512]
        w1p = np.ascontiguousarray(
            w1pair.reshape(N_PAIR, NCK, 128, 128).transpose(2, 1, 0, 3).reshape(128, -1)
        )
        w3p = np.ascontiguousarray(
            w3pair.reshape(N_PAIR, NCK, 128, 128).transpose(2, 1, 0, 3).reshape(128, -1)
        )
        w2p = np.ascontiguousarray(w2pair.transpose(1, 0, 2).reshape(128, -1))

        rows_sh = np.zeros((2, NTOK), dtype=np.float32)
        rows_sh[:, 128 * c : 128 * (c + 1)] = 1.0

        in_maps.append(
            {
                "xt": xt,
                "gwt": gwt_c,
                "biasb": biasb,
                "w1p": w1p,
                "w3p": w3p,
                "w2p": w2p,
                "rows_sh": rows_sh,
                "esel": esel,
                "ident": ident,
            }
        )
    return in_maps


_NC_CACHE = None


def kernel(x, g_w, gate_bias, w1, w2, w3):
    global _NC_CACHE
    if _NC_CACHE is None:
        _NC_CACHE = build_nc()
    nc = _NC_CACHE
    in_maps = make_core_inputs(x, g_w, gate_bias, w1, w2, w3)
    res = run_bass_kernel_spmd(nc, in_maps, list(range(N_CORES)))
    out = np.zeros((NTOK, DIM), dtype=np.float32)
    for r in res.results:
        out += r["pout"]
    return out.reshape(B, T, DIM)


# revision 21
# speedup vs baseline: 1.1327x; 1.1327x over previous
"""DeepSeek-MoE block on 8 Trainium2 NeuronCores (Bass/Tile).

Sharding: expert-parallel. Each core owns 8 of the 64 routed experts plus a
slice of the 2 shared experts. Every core computes the full gate
(softmax + top-6 threshold) for all 1024 tokens, then runs a masked-dense FFN
over its experts: the per-(token, expert) combine weight is zero for
unselected experts, so no token dispatch is needed. Core outputs are partial
sums; the host unshard is a sum over the 8 partials.

Fixed problem shapes (hardcoded per the harness contract):
  x [2, 512, 512] f32, g_w [64, 512], gate_bias [64],
  w1/w3 [66, 512, 64], w2 [66, 64, 512]; 2 shared + 64 routed, top-6.
"""

import sys

import numpy as np

if "/opt/trn_rl_repo" not in sys.path:
    sys.path.insert(0, "/opt/trn_rl_repo")

import concourse.bass as bass
import concourse.mybir as mybir
import concourse.tile as tile
from concourse import bacc
from concourse.bass_utils import run_bass_kernel_spmd

DIM = 512
INTER = 64
N_SHARED = 2
N_ROUTED = 64
TOPK = 6
B, T = 2, 512
NTOK = B * T                 # 1024 tokens
N_CORES = 8
EXP_PER_CORE = N_ROUTED // N_CORES   # 8 routed experts per core
N_SLOT = EXP_PER_CORE + 2            # + 2 shared-expert slots
N_PAIR = N_SLOT // 2                 # 5 expert pairs
N_TILE = NTOK // 128                 # 8 token tiles of 128
ST = 256                             # supertile token width for the FFN
N_ST = NTOK // ST                    # 4 supertiles
NCK = DIM // 128                     # 4 contraction chunks
HALF = NTOK // 2                     # gate processed in 2 token-halves
HT = 4                               # token tiles per half

F32 = mybir.dt.float32
F32R = mybir.dt.float32r
AF = mybir.ActivationFunctionType
ALU = mybir.AluOpType


def build_nc(silu_native=True):
    """Build the single-core Bass program (SPMD across 8 cores).

    silu_native=False lowers SiLU as Sigmoid+mult (CoreSim has no Silu).
    """
    nc = bacc.Bacc("TRN2", target_bir_lowering=False, debug=False)

    # ---- DRAM I/O (per-core values supplied by the host) ----
    # xt: [128, ck*1024] chunk-major per partition (host pre-layouts)
    xt_d = nc.dram_tensor("xt", [128, NCK * NTOK], F32, kind="ExternalInput")
    gwt_d = nc.dram_tensor("gwt", [128, NCK * N_ROUTED], F32, kind="ExternalInput")
    biasb_d = nc.dram_tensor("biasb", [128, HT * N_ROUTED], F32, kind="ExternalInput")
    w1p_d = nc.dram_tensor("w1p", [128, NCK * N_PAIR * 128], F32R, kind="ExternalInput")
    w3p_d = nc.dram_tensor("w3p", [128, NCK * N_PAIR * 128], F32R, kind="ExternalInput")
    w2p_d = nc.dram_tensor("w2p", [128, N_PAIR * DIM], F32R, kind="ExternalInput")
    rows_sh_d = nc.dram_tensor("rows_sh", [2, NTOK], F32, kind="ExternalInput")
    esel_d = nc.dram_tensor("esel", [N_ROUTED + 2, N_PAIR * 128], F32R, kind="ExternalInput")
    ident_d = nc.dram_tensor("ident", [128, 128], F32, kind="ExternalInput")
    pout_d = nc.dram_tensor("pout", [NTOK, DIM], F32, kind="ExternalOutput")

    with tile.TileContext(nc) as tc:
        with (
            tc.tile_pool(name="const", bufs=1) as cpool,
            tc.tile_pool(name="gate", bufs=1) as gpool,
            tc.tile_pool(name="act", bufs=4) as apool,
            tc.tile_pool(name="psA", bufs=2, space="PSUM") as psA,
            tc.tile_pool(name="psO", bufs=1, space="PSUM") as psO,
        ):
            # ---- persistent SBUF loads, one ordered queue for the big ones ----
            xt_sb = cpool.tile([128, NCK * NTOK], F32, tag="xt")
            nc.sync.dma_start(xt_sb[:], xt_d.ap())
            gwt_sb = cpool.tile([128, NCK * N_ROUTED], F32, tag="gwt")
            nc.sync.dma_start(gwt_sb[:], gwt_d.ap())
            xtr_sb = cpool.tile([128, NCK * NTOK], F32R, tag="xtr")
            nc.sync.dma_start(xtr_sb[:], xt_d.ap().bitcast(F32R))
            w1p_sb = cpool.tile([128, NCK * N_PAIR * 128], F32R, tag="w1p")
            nc.sync.dma_start(w1p_sb[:], w1p_d.ap())
            w3p_sb = cpool.tile([128, NCK * N_PAIR * 128], F32R, tag="w3p")
            nc.sync.dma_start(w3p_sb[:], w3p_d.ap())
            w2p_sb = cpool.tile([128, N_PAIR * DIM], F32R, tag="w2p")
            nc.sync.dma_start(w2p_sb[:], w2p_d.ap())

            ident_sb = cpool.tile([128, 128], F32, tag="ident")
            nc.gpsimd.dma_start(ident_sb[:], ident_d.ap())
            biasb_sb = cpool.tile([128, HT * N_ROUTED], F32, tag="biasb")
            nc.gpsimd.dma_start(biasb_sb[:], biasb_d.ap())
            esel_sb = cpool.tile([N_ROUTED + 2, N_PAIR * 128], F32R, tag="esel")
            nc.gpsimd.dma_start(esel_sb[:], esel_d.ap())

            # combine-weight rows: 0..63 routed-transposed (only 0..7 selected
            # by esel), 64..65 the shared-expert token masks
            wt_sb = gpool.tile([N_ROUTED + 2, NTOK], F32R, tag="wt")
            nc.gpsimd.dma_start(
                wt_sb[N_ROUTED : N_ROUTED + 2, :], rows_sh_d.ap().bitcast(F32R)
            )

            def gate_half(h):
                """Compute combine-weight rows of wt_sb for tokens
                [h*512, (h+1)*512)."""
                base = h * HALF
                # scoresT [64, 512] fp32 (N=512 moving)
                scT_ps = psA.tile([N_ROUTED, HALF], F32, tag="h1", name=f"scTps{h}")
                for ck in range(NCK):
                    nc.tensor.matmul(
                        scT_ps[:],
                        gwt_sb[:, ck * N_ROUTED : (ck + 1) * N_ROUTED],
                        xt_sb[:, ck * NTOK + base : ck * NTOK + base + HALF],
                        start=(ck == 0),
                        stop=(ck == NCK - 1),
                    )
                scT = gpool.tile([N_ROUTED, HALF], F32, tag=f"scT{h}")
                nc.scalar.copy(scT[:], scT_ps[:])

                # transpose to token-partition layout [128, tile, 64]
                scores = gpool.tile([128, HT * N_ROUTED], F32, tag=f"scores{h}")
                for tt in range(HT):
                    tps = psA.tile([128, N_ROUTED], F32, tag="h3", name=f"tps{h}_{tt}")
                    nc.tensor.transpose(
                        tps[:], scT[:, tt * 128 : (tt + 1) * 128], ident_sb[0:64, 0:64]
                    )
                    nc.scalar.copy(scores[:, tt * N_ROUTED : (tt + 1) * N_ROUTED], tps[:])

                def b3(t):
                    return t.rearrange("p (t e) -> p t e", e=N_ROUTED)

                def bc(t):
                    return t.unsqueeze(-1).to_broadcast([128, HT, N_ROUTED])

                g = lambda tag: gpool.tile([128, HT * N_ROUTED], F32, tag=f"{tag}{h}", name=f"{tag}{h}")
                s = lambda tag: gpool.tile([128, HT], F32, tag=f"{tag}{h}", name=f"{tag}{h}")

                rmax = s("rmax")
                nc.vector.tensor_reduce(rmax[:], b3(scores[:]), axis=mybir.AxisListType.X, op=ALU.max)
                shifted = g("shifted")
                nc.vector.tensor_tensor(b3(shifted[:]), b3(scores[:]), bc(rmax[:]), op=ALU.subtract)
                exps = g("exps")
                nc.scalar.activation(exps[:], shifted[:], AF.Exp)
                rsum = s("rsum")
                nc.vector.tensor_reduce(rsum[:], b3(exps[:]), axis=mybir.AxisListType.X, op=ALU.add)
                rinv = s("rinv")
                nc.vector.reciprocal(rinv[:], rsum[:])
                probs = g("probs")
                nc.vector.tensor_tensor(b3(probs[:]), b3(exps[:]), bc(rinv[:]), op=ALU.mult)

                biased = g("biased")
                nc.vector.tensor_tensor(biased[:], probs[:], biasb_sb[:], op=ALU.add)
                work = g("work")
                nc.scalar.copy(work[:], biased[:])

                m = None
                for it in range(TOPK):
                    m = s(f"m{it}_")
                    nc.vector.tensor_reduce(m[:], b3(work[:]), axis=mybir.AxisListType.X, op=ALU.max)
                    if it < TOPK - 1:
                        keep = g("keep")
                        nc.vector.tensor_tensor(b3(keep[:]), b3(work[:]), bc(m[:]), op=ALU.is_lt)
                        nc.vector.tensor_tensor(work[:], work[:], keep[:], op=ALU.mult)

                mask = g("mask")
                nc.vector.tensor_tensor(b3(mask[:]), b3(biased[:]), bc(m[:]), op=ALU.is_ge)
                wcomb = g("wcomb")
                nc.vector.tensor_tensor(wcomb[:], probs[:], mask[:], op=ALU.mult)

                for tt in range(HT):
                    wtp = psA.tile([N_ROUTED, 128], F32, tag="h3", name=f"wtp{h}_{tt}")
                    nc.tensor.transpose(
                        wtp[:], wcomb[:, tt * N_ROUTED : (tt + 1) * N_ROUTED], ident_sb[:]
                    )
                    nc.scalar.copy(
                        wt_sb[0:N_ROUTED, base + tt * 128 : base + (tt + 1) * 128], wtp[:]
                    )

            def ffn_supertile(st):
                t0 = st * ST
                outp = [
                    psO.tile([128, DIM], F32, name=f"outp{st}_{s}", tag=f"out{s}")
                    for s in range(ST // 128)
                ]
                for p in range(N_PAIR):
                    h1 = psA.tile([128, ST], F32, tag="h1")
                    h3 = psA.tile([128, ST], F32, tag="h3")
                    wb = psA.tile([128, ST], F32, tag="wb")
                    for ck in range(NCK):
                        xck = xtr_sb[:, ck * NTOK + t0 : ck * NTOK + t0 + ST]
                        nc.tensor.matmul(
                            h1[:],
                            w1p_sb[:, (ck * N_PAIR + p) * 128 : (ck * N_PAIR + p + 1) * 128],
                            xck,
                            start=(ck == 0),
                            stop=(ck == NCK - 1),
                        )
                        nc.tensor.matmul(
                            h3[:],
                            w3p_sb[:, (ck * N_PAIR + p) * 128 : (ck * N_PAIR + p + 1) * 128],
                            xck,
                            start=(ck == 0),
                            stop=(ck == NCK - 1),
                        )
                    nc.tensor.matmul(
                        wb[:],
                        esel_sb[:, p * 128 : (p + 1) * 128],
                        wt_sb[:, t0 : t0 + ST],
                        start=True,
                        stop=True,
                    )
                    # ACT evacuates h1/h3 so PE can run far ahead of the gate
                    silu = apool.tile([128, ST], F32, tag="silu")
                    if silu_native:
                        nc.scalar.activation(silu[:], h1[:], AF.Silu)
                    else:
                        # CoreSim path: silu = h1 * sigmoid(h1)
                        sg = apool.tile([128, ST], F32, tag="sg")
                        nc.scalar.activation(sg[:], h1[:], AF.Sigmoid)
                        h1s = apool.tile([128, ST], F32, tag="h1s")
                        nc.scalar.copy(h1s[:], h1[:])
                        nc.vector.tensor_tensor(silu[:], sg[:], h1s[:], op=ALU.mult)
                    h3s = apool.tile([128, ST], F32, tag="h3s")
                    nc.scalar.copy(h3s[:], h3[:])
                    prod = apool.tile([128, ST], F32, tag="prod", bufs=10)
                    nc.gpsimd.tensor_tensor(prod[:], silu[:], h3s[:], op=ALU.mult)
                    aT = apool.tile([128, ST], F32R, tag="aT")
                    nc.vector.tensor_tensor(aT[:], prod[:], wb[:], op=ALU.mult)
                    for s in range(ST // 128):
                        nc.tensor.matmul(
                            outp[s][:],
                            aT[:, s * 128 : (s + 1) * 128],
                            w2p_sb[:, p * DIM : (p + 1) * DIM],
                            start=(p == 0),
                            stop=(p == N_PAIR - 1),
                        )
                for s in range(ST // 128):
                    osb = apool.tile([128, DIM], F32, tag="osb")
                    nc.scalar.copy(osb[:], outp[s][:])
                    nc.sync.dma_start(
                        pout_d.ap()[t0 + s * 128 : t0 + (s + 1) * 128, :], osb[:]
                    )

            gate_half(0)
            ffn_supertile(0)
            ffn_supertile(1)
            gate_half(1)
            ffn_supertile(2)
            ffn_supertile(3)

    nc.compile()
    return nc


def make_core_inputs(x, g_w, gate_bias, w1, w2, w3):
    """Host-side sharding/layout prep. Returns list of 8 per-core input maps."""
    x = np.ascontiguousarray(np.asarray(x, dtype=np.float32)).reshape(NTOK, DIM)
    g_w = np.asarray(g_w, dtype=np.float32)
    gate_bias = np.asarray(gate_bias, dtype=np.float32)
    w1 = np.asarray(w1, dtype=np.float32)
    w2 = np.asarray(w2, dtype=np.float32)
    w3 = np.asarray(w3, dtype=np.float32)

    # xt host layout: [128 partitions, ck*1024] with xt[p, ck*1024+t] = x[t, ck*128+p]
    xt = np.ascontiguousarray(
        x.T.reshape(NCK, 128, NTOK).transpose(1, 0, 2).reshape(128, NCK * NTOK)
    )
    bias_shift = gate_bias - gate_bias.min() + 1.0      # keep biased scores > 0
    ident = np.eye(128, dtype=np.float32)
    # esel[k, p*128 + j] selects wt row k into broadcast partitions j of pair p:
    # pair p < 4 -> routed rows (2p, 2p+1); pair 4 -> shared rows (64, 65)
    esel = np.zeros((N_ROUTED + 2, N_PAIR * 128), dtype=np.float32)
    for p in range(N_PAIR):
        r0 = 2 * p if p < N_PAIR - 1 else N_ROUTED
        esel[r0, p * 128 : p * 128 + 64] = 1.0
        esel[r0 + 1, p * 128 + 64 : (p + 1) * 128] = 1.0

    in_maps = []
    for c in range(N_CORES):
        mine = list(range(EXP_PER_CORE * c, EXP_PER_CORE * (c + 1)))
        perm = mine + [e for e in range(N_ROUTED) if e not in mine]
        # gwt host layout [128, ck*64]: gwt[p, ck*64+e] = g_w[perm[e], ck*128+p]
        gwt_c = np.ascontiguousarray(
            g_w[perm].T.reshape(NCK, 128, N_ROUTED).transpose(1, 0, 2).reshape(128, -1)
        )
        biasb = np.tile(bias_shift[perm], (128, HT))      # [128, 256]

        # expert slots: 8 routed (global idx 2+e) then the 2 shared experts
        slots = [2 + e for e in mine] + [0, 1]
        w1s = w1[slots]                                  # [10, 512, 64]
        w3s = w3[slots]
        w2s = w2[slots]                                  # [10, 64, 512]
        # pair p = slots (2p, 2p+1) concatenated along the inter axis
        w1pair = np.stack(
            [np.concatenate([w1s[2 * p], w1s[2 * p + 1]], axis=1) for p in range(N_PAIR)]
        )  # [5, 512, 128]
        w3pair = np.stack(
            [np.concatenate([w3s[2 * p], w3s[2 * p + 1]], axis=1) for p in range(N_PAIR)]
        )
        w2pair = np.stack(
            [np.concatenate([w2s[2 * p], w2s[2 * p + 1]], axis=0) for p in range(N_PAIR)]
        )  # [5, 128, 512]

        # SBUF layouts: w1p [128p, ck, pair, 128], w2p [128p, pair*512]
        w1p = np.ascontiguousarray(
            w1pair.reshape(N_PAIR, NCK, 128, 128).transpose(2, 1, 0, 3).reshape(128, -1)
        )
        w3p = np.ascontiguousarray(
            w3pair.reshape(N_PAIR, NCK, 128, 128).transpose(2, 1, 0, 3).reshape(128, -1)
        )
        w2p = np.ascontiguousarray(w2pair.transpose(1, 0, 2).reshape(128, -1))

        rows_sh = np.zeros((2, NTOK), dtype=np.float32)
        rows_sh[:, 128 * c : 128 * (c + 1)] = 1.0

        in_maps.append(
            {
                "xt": xt,
                "gwt": gwt_c,
                "biasb": biasb,
                "w1p": w1p,
                "w3p": w3p,
                "w2p": w2p,
                "rows_sh": rows_sh,
                "esel": esel,
                "ident": ident,
            }
        )
    return in_maps


_NC_CACHE = None


def kernel(x, g_w, gate_bias, w1, w2, w3):
    global _NC_CACHE
    if _NC_CACHE is None:
        _NC_CACHE = build_nc()
    nc = _NC_CACHE
    in_maps = make_core_inputs(x, g_w, gate_bias, w1, w2, w3)
    res = run_bass_kernel_spmd(nc, in_maps, list(range(N_CORES)))
    out = np.zeros((NTOK, DIM), dtype=np.float32)
    for r in res.results:
        out += r["pout"]
    return out.reshape(B, T, DIM)
